# revision 21
# baseline (speedup 1.0000x reference)
"""Trainium2 Bass kernel for nn_MultiHeadFast (multi-head attention with
softmax over the QUERY axis).

Math (faithful to the reference):
  qkv = x @ Ws;  per (b,h):  S[q,k] = Q.K^T,  causal mask k<=q,
  P = softmax_over_q(S * T^-0.5),  out = P @ V.

Layout strategy (no on-device transposes at all):
  * The host passes x^T already transposed and cast to bf16, laid out as
    [128 e-partitions, 8 e-blocks, 4096 tokens], plus the per-core Ws column
    slice in the matching [128, 8, 384] layout.  This removes the 256 PE
    transposes + PSUM evacuation copies a naive layout spends most of its
    time on.
  * Q^T/K^T (features on partitions) come straight from matmuls with Ws as
    lhsT.  V is computed token-natural (tokens on partitions) by swapping
    operands: out[tok,feat] = xT_tile.T @ Wv_block — no V transpose.
  * S is computed TRANSPOSED (S^T[k,q], keys on partitions) so the q-axis
    softmax is a free-axis reduction and S^T feeds out^T = V^T @ P directly.
    The per-key normalizer folds into V rows before the PV matmul.
  * The output is written transposed (out^T[feat, q]) to DRAM; the host
    un-transposes when gathering — no output transposes on device.
  * All PSUM flows through a 2x[128,1024] ring plus 4 PV accumulator banks
    (= exactly 8 banks).  Batch-1 QKV work is hand-interleaved into
    batch-0's attention k-loop so the ACT exp stream (the critical path)
    never starves.

Sharding: tensor-parallel over heads.  Core c owns heads {2c, 2c+1}; no
collectives.  bf16 with fp32 accumulation (~4e-3 L2 vs fp32 reference).
"""

import numpy as np
import ml_dtypes
from contextlib import ExitStack

import concourse.bass as bass
import concourse.mybir as mybir
import concourse.tile as tile
from concourse import bacc
from concourse.bass_utils import run_bass_kernel_spmd

B, T, E = 2, 2048, 1024
H, D = 16, 64
NCORES = 8
HPC = H // NCORES            # heads per core = 2
FPC = HPC * D                # feature cols per core per Q/K/V = 128
P = 128
NT = B * T                   # 4096 tokens total
EK = E // P                  # 8 contraction blocks for QKV
NSLAB = T // 512             # 4 query slabs per batch
KTILES = T // P              # 16 key tiles per batch
DT = mybir.dt.bfloat16
F32 = mybir.dt.float32
BF16 = ml_dtypes.bfloat16
SCALE = float(T) ** -0.5
NEG = -1e30


def build_kernel():
    nc = bacc.Bacc("TRN2", target_bir_lowering=False, debug=False)
    xt_dram = nc.dram_tensor("xt", (P, EK, NT), DT, kind="ExternalInput")
    w_dram = nc.dram_tensor("wsl", (P, EK, 3 * FPC), DT, kind="ExternalInput")
    out_dram = nc.dram_tensor("out", (B, FPC, T), F32, kind="ExternalOutput")

    with tile.TileContext(nc) as tc, ExitStack() as ctx:
        const = ctx.enter_context(tc.tile_pool(name="const", bufs=1))
        xtp = ctx.enter_context(tc.tile_pool(name="xtp", bufs=1))
        qkvp = ctx.enter_context(tc.tile_pool(name="qkvp", bufs=1))
        strips = ctx.enter_context(tc.tile_pool(name="strips", bufs=4))
        small = ctx.enter_context(tc.tile_pool(name="small", bufs=8))
        outp = ctx.enter_context(tc.tile_pool(name="outp", bufs=2))
        ps = ctx.enter_context(tc.tile_pool(name="ps", bufs=2, space="PSUM"))

        # ---- constants ----
        zeros_bf = const.tile([P, P], DT, name="zeros_bf")
        nc.gpsimd.memset(zeros_bf[:], 0.0)
        # diagmask[p, f] = 0 if f >= p else NEG   (keys on partitions, q free)
        diagmask = const.tile([P, P], F32, name="diagmask")
        nc.gpsimd.memset(diagmask[:], 0.0)
        nc.gpsimd.affine_select(
            out=diagmask[:],
            in_=diagmask[:],
            compare_op=mybir.AluOpType.is_ge,
            fill=NEG,
            base=0,
            pattern=[[1, P]],
            channel_multiplier=-1,
        )

        # ---- inputs (already bf16 + transposed on host) ----
        wsl = qkvp.tile([P, EK, 3 * FPC], DT, name="wsl")
        nc.sync.dma_start(wsl[:], w_dram[:])
        xT = xtp.tile([P, EK, NT], DT, name="xT")
        for s in range(NT // 1024):
            nc.sync.dma_start(
                xT[:, :, 1024 * s : 1024 * (s + 1)],
                xt_dram[:, :, 1024 * s : 1024 * (s + 1)],
            )

        qt = qkvp.tile([P, NT], DT, name="qt")
        kt = qkvp.tile([P, NT], DT, name="kt")
        v_nat = qkvp.tile([P, B * KTILES, FPC], DT, name="v_nat")

        def ring(name):
            return ps.tile([P, 1024], F32, tag="ring", bufs=2, name=name)

        def qk_group(b, c, m):
            """Q (m=0) or K (m=1) projection for one 512-token chunk."""
            t0 = b * T + 512 * c
            slot = ring(f"qk{b}{c}{m}")
            for e in range(EK):
                nc.tensor.matmul(
                    slot[:, 0:512],
                    lhsT=wsl[:, e, m * FPC : (m + 1) * FPC],
                    rhs=xT[:, e, t0 : t0 + 512],
                    start=(e == 0),
                    stop=(e == EK - 1),
                )
            dst = qt if m == 0 else kt
            nc.vector.tensor_copy(dst[:, t0 : t0 + 512], slot[:, 0:512])

        def v_group(b, w):
            """V projection, token-natural, for one 128-token tile."""
            t0 = b * T + P * w
            slot = ring(f"v{b}{w}")
            for e in range(EK):
                nc.tensor.matmul(
                    slot[:, 0:FPC],
                    lhsT=xT[:, e, t0 : t0 + P],
                    rhs=wsl[:, e, 2 * FPC : 3 * FPC],
                    start=(e == 0),
                    stop=(e == EK - 1),
                )
            nc.vector.tensor_copy(v_nat[:, b * KTILES + w, :], slot[:, 0:FPC])

        def attention(b, inject):
            """Attention for batch b; inject[k] = extra work emitted at iter k."""
            pv_ps = [
                ps.tile([P, 512], F32, tag="pvb", bufs=4, name=f"pv{b}{j}")
                for j in range(NSLAB)
            ]
            # Zero-init each PV bank so every real PV matmul accumulates with
            # start=False (uniform has_written under either bank semantics).
            for j in range(NSLAB):
                nc.tensor.matmul(
                    pv_ps[j][:],
                    lhsT=zeros_bf[:],
                    rhs=wsl.rearrange("p e f -> p (e f)")[:, 0:512],
                    start=True,
                    stop=False,
                    skip_group_check=True,
                )

            def chunk_mms(k, hh, strip, coff, cw):
                """S^T matmuls + mask + exp for one <=1024-col chunk."""
                j0 = k // 4
                q0 = 512 * j0
                dead = P * k - q0
                sps = ring(f"sps{b}{k}{hh}{coff}")
                lo = dead if coff == 0 else 0
                for s0 in range(0, cw, 512):
                    a = max(s0, lo)
                    hi = min(s0 + 512, cw)
                    if a >= hi:
                        continue
                    qs = b * T + q0 + coff + a
                    nc.tensor.matmul(
                        sps[:, a:hi],
                        lhsT=kt[hh * D : (hh + 1) * D, b * T + k * P : b * T + k * P + P],
                        rhs=qt[hh * D : (hh + 1) * D, qs : qs + (hi - a)],
                        start=True,
                        stop=True,
                    )
                acc = small.tile([P, 1], F32, tag="acc", name="acc")
                if coff == 0:
                    nc.vector.tensor_add(
                        sps[:, dead : dead + P], sps[:, dead : dead + P], diagmask[:]
                    )
                    nc.scalar.activation(
                        strip[:, 0 : cw - dead],
                        sps[:, dead:cw],
                        mybir.ActivationFunctionType.Exp,
                        scale=SCALE,
                        accum_out=acc[:],
                    )
                else:
                    nc.scalar.activation(
                        strip[:, coff - dead : coff - dead + cw],
                        sps[:, :cw],
                        mybir.ActivationFunctionType.Exp,
                        scale=SCALE,
                        accum_out=acc[:],
                    )
                return acc

            def finish_head(k, hh, partials):
                if len(partials) == 1:
                    ssum = partials[0]
                else:
                    ssum = small.tile([P, 1], F32, tag="acc", name="ssum")
                    nc.vector.tensor_add(ssum[:], partials[0][:], partials[1][:])
                rsum = small.tile([P, 1], F32, tag="acc", name="rsum")
                nc.vector.reciprocal(rsum[:], ssum[:])
                vp = small.tile([P, D], DT, tag="vp", name="vp")
                nc.vector.tensor_scalar_mul(
                    vp[:], v_nat[:, b * KTILES + k, hh * D : (hh + 1) * D], rsum[:]
                )
                return vp

            def pv_head(k, hh, strip, vp):
                j0 = k // 4
                dead = P * k - 512 * j0
                for j in range(j0, NSLAB):
                    if j == j0 and dead > 0:
                        o, w, soff = dead, 512 - dead, 0
                    else:
                        o, w, soff = 0, 512, 512 * j - P * k
                    nc.tensor.matmul(
                        pv_ps[j][hh * D : (hh + 1) * D, o : o + w],
                        lhsT=vp[:],
                        rhs=strip[:, soff : soff + w],
                        start=False,
                        stop=(k == 4 * j + 3 and hh == HPC - 1),
                        skip_group_check=True,
                    )

            def evac(j):
                osb = outp.tile([P, 512], F32, tag="osb", bufs=2, name=f"osb{b}{j}")
                nc.vector.tensor_copy(osb[:], pv_ps[j][:])
                nc.sync.dma_start(out_dram[b, :, 512 * j : 512 * (j + 1)], osb[:])

            # software pipeline: per k-tile, S matmuls for both heads feed the
            # ACT exp stream; PV of k-1 and injected QKV work fill the PE.
            prev = {}
            for k in range(KTILES):
                j0 = k // 4
                L = T - 512 * j0
                strip_k = {}
                parts = {0: [], 1: []}
                for hh in range(HPC):
                    strip_k[hh] = strips.tile([P, T], DT, tag="strip", name=f"s{hh}")
                coff = 0
                while coff < L:
                    cw = min(1024, L - coff)
                    for hh in range(HPC):
                        parts[hh].append(chunk_mms(k, hh, strip_k[hh], coff, cw))
                    coff += cw
                for hh in range(HPC):
                    vp = finish_head(k, hh, parts[hh])
                    if k > 0:
                        pv_head(k - 1, hh, *prev[hh])
                    prev[hh] = (strip_k[hh], vp)
                for fn in inject.get(k, ()):
                    fn()
                if k >= 4 and k % 4 == 0:
                    evac(k // 4 - 1)
            for hh in range(HPC):
                pv_head(KTILES - 1, hh, *prev[hh])
            evac(NSLAB - 1)

        # ---- phase A0: batch-0 QKV (ACT has nothing to do yet) ----
        qk_group(0, 0, 0)
        qk_group(0, 0, 1)
        v_group(0, 0)
        v_group(0, 1)
        qk_group(0, 1, 0)
        qk_group(0, 1, 1)
        v_group(0, 2)
        v_group(0, 3)
        qk_group(0, 2, 0)
        qk_group(0, 2, 1)
        qk_group(0, 3, 0)
        qk_group(0, 3, 1)

        # ---- batch-0 attention, with batch-1 QKV + late batch-0 V injected ----
        injB0 = {}
        for k in range(2, 14):  # v_nat(0) tiles 4..15, two iters ahead of use
            injB0.setdefault(k, []).append(lambda t=k + 2: v_group(0, t))
        for k in range(8):      # batch-1 Q/K projections
            c, m = divmod(k, 2)
            injB0.setdefault(k, []).append(lambda c=c, m=m: qk_group(1, c, m))
        injB0.setdefault(14, []).append(lambda: v_group(1, 0))
        injB0.setdefault(15, []).append(lambda: v_group(1, 1))
        attention(0, injB0)

        # ---- batch-1 attention, with remaining batch-1 V injected ----
        injB1 = {}
        for k in range(14):
            injB1.setdefault(k, []).append(lambda t=k + 2: v_group(1, t))
        attention(1, injB1)

    nc.compile()
    return nc


def make_in_maps(x: np.ndarray, Ws: np.ndarray):
    """Host-side shard + layout prep: x^T and per-core Ws slices, bf16,
    laid out [128 e-partitions, 8 e-blocks, cols] to match the kernel."""
    x2 = x.reshape(NT, E)
    xt = (
        np.ascontiguousarray(x2.T.reshape(EK, P, NT).transpose(1, 0, 2))
        .astype(BF16)
    )
    in_maps = []
    for c in range(NCORES):
        cols = np.concatenate(
            [
                Ws[:, c * FPC : (c + 1) * FPC],
                Ws[:, E + c * FPC : E + (c + 1) * FPC],
                Ws[:, 2 * E + c * FPC : 2 * E + (c + 1) * FPC],
            ],
            axis=1,
        )
        wsl = np.ascontiguousarray(
            cols.reshape(EK, P, 3 * FPC).transpose(1, 0, 2)
        ).astype(BF16)
        in_maps.append({"xt": xt, "wsl": wsl})
    return in_maps


def gather_out(results) -> np.ndarray:
    """Assemble per-core transposed outputs into the full (B, T, H*D)."""
    out = np.empty((B, T, H * D), np.float32)
    for c in range(NCORES):
        out[:, :, c * FPC : (c + 1) * FPC] = results[c]["out"].transpose(0, 2, 1)
    return out


_NC_CACHE = None


def kernel(x: np.ndarray, Ws: np.ndarray) -> np.ndarray:
    global _NC_CACHE
    if _NC_CACHE is None:
        _NC_CACHE = build_kernel()
    nc = _NC_CACHE

    in_maps = make_in_maps(
        np.asarray(x, np.float32), np.asarray(Ws, np.float32)
    )
    res = run_bass_kernel_spmd(nc, in_maps, core_ids=list(range(NCORES)))
    return gather_out(res.results)
